# revision 12
# baseline (speedup 1.0000x reference)
"""Trainium2 Bass kernel for nn_MultiHeadAttention_59614146068609.

Sharding: 8 cores = 2 batches x 4 head-groups (4 heads each). Each core
projects q/k/v for its batch with its head-slice of Wq/Wk/Wv, runs
causal+padded attention for its 4 heads, applies its row-slice of Wo,
and writes a partial [D, S] fp16 output. The host sums the 4 partials
per batch and adds bo.

Key structure (v2):
- q/k stored packed 2-heads-per-128-partitions; scores matmuls are
  row-tiled (tile_position (0,0)/(64,0)) so both heads' QK^T run
  concurrently on the PE sub-arrays.
- scores for a head-pair land in one [128,1024] 2-bank PSUM tile; a
  single wide instruction computes exp for both heads at once.
- exp alternates between the Scalar (ACT spline exp) and Vector engine
  (Schraudolph bit-trick exp: round(s*1477.32+15315) as uint16 = fp16
  bits of ~e^s, +-3% sawtooth) to double softmax throughput.
- causal masking inside diagonal 128-blocks is done by gpsimd
  affine_select zeroing p after exp; key padding is handled entirely by
  zeroed V rows + a masked ones-column (no bias in the exp at all).
- q/k/v biases are folded into the projection matmuls as K=1 matmuls
  against a ones/valid row.
- softmax normalization: the ones-column of V gives the denominator at
  PSUM partition 64; copy to SBUF (ScalarE), reciprocal (DVE),
  partition_broadcast (gpsimd), one multiply per head half.
"""

import numpy as np

S = 2048
B = 2
D = 1024
H = 16
DK = 64
N_CORES = 8
GROUPS = N_CORES // B          # head groups per batch = 4
HPG = H // GROUPS              # heads per group = 4
OC = HPG * DK                  # per-core projected dim = 256
OT = OC // 128                 # head-pairs per core = 2
IT = D // 128                  # contraction tiles = 8
SC = S // 512                  # sequence chunks of 512 = 4

A_EXP = float(1024.0 / np.log(2.0))    # Schraudolph scale
B_EXP = 15360.0 - 45.0                 # Schraudolph bias (calibrated)

_cache = {}


def _build_nc(kb_cap):
    import concourse.bacc as bacc
    import concourse.bass as bass
    import concourse.mybir as mybir
    import concourse.tile as tile
    from concourse import library_config

    F32 = mybir.dt.float32
    F16 = mybir.dt.float16
    U16 = mybir.dt.uint16
    Exp = mybir.ActivationFunctionType.Exp
    Mult = mybir.AluOpType.mult
    Add = mybir.AluOpType.add
    IsGe = mybir.AluOpType.is_ge
    PSUM = bass.MemorySpace.PSUM

    ksc = -(-kb_cap * 128 // 512)      # 512-chunks of kT to project
    SK = ksc * 512
    vchunks = [list(range(s, min(s + 8, kb_cap))) for s in range(0, kb_cap, 8)]

    nc = bacc.Bacc("TRN2", target_bir_lowering=False, debug=False)

    xq = nc.dram_tensor("xq", [D, S], F16, kind="ExternalInput")
    xk = nc.dram_tensor("xk", [D, S], F16, kind="ExternalInput")
    xv = nc.dram_tensor("xv", [D, S], F16, kind="ExternalInput")
    wv_d = nc.dram_tensor("wv_d", [128, IT * OC], F16, kind="ExternalInput")
    # wq | wk | wo concatenated: one DMA issue
    wrest = nc.dram_tensor("wrest", [128, 2 * IT * OC + OT * D], F16, kind="ExternalInput")
    # bias [3*OC] | vrow [S] | ones [512] concatenated
    brow = nc.dram_tensor("brow", [1, 3 * OC + S + 512], F16, kind="ExternalInput")
    vones = nc.dram_tensor("vones", [128, kb_cap * HPG], F16, kind="ExternalInput")
    out_t = nc.dram_tensor("out_t", [D, S], F16, kind="ExternalOutput")

    with tile.TileContext(nc) as tc, nc.allow_low_precision(
        reason="fp16 compute + approx exp on part of softmax; validated vs reference"
    ):
        with (
            tc.tile_pool(name="persist", bufs=1) as pp,
            tc.tile_pool(name="xs", bufs=3) as xs,
            tc.tile_pool(name="scratch", bufs=2) as scr,
        ):
            nc.gpsimd.load_library(library_config.attn)

            t_wv_ = pp.tile([128, IT, OC], F16)
            t_wrest = pp.tile([128, 2 * IT * OC + OT * D], F16)
            t_brow = pp.tile([1, 3 * OC + S + 512], F16)
            t_qT = pp.tile([128, OT, S], F16)
            t_kT = pp.tile([128, OT, SK], F16)
            t_V = pp.tile([128, kb_cap, HPG, DK + 1], F16)
            t_OT = pp.tile([128, OT, S], F16)

            t_wv = t_wv_[:, :, :]
            t_wq = t_wrest[:, 0 : IT * OC].rearrange("p (i o) -> p i o", i=IT)
            t_wk = t_wrest[:, IT * OC : 2 * IT * OC].rearrange(
                "p (i o) -> p i o", i=IT
            )
            t_wo = t_wrest[:, 2 * IT * OC :].rearrange("p (j d) -> p j d", j=OT)
            t_bias = t_brow[:, 0 : 3 * OC].rearrange("p (t o) -> p t o", t=3)
            t_vrow = t_brow[:, 3 * OC : 3 * OC + S]
            t_ones = t_brow[:, 3 * OC + S :]

            nc.scalar.dma_start(
                out=t_wv_, in_=wv_d[:].rearrange("p (i o) -> p i o", i=IT)
            )
            nc.scalar.dma_start(out=t_brow, in_=brow[:])
            nc.scalar.dma_start(out=t_wrest, in_=wrest[:])
            t_vo = pp.tile([128, kb_cap * HPG], F16)
            nc.scalar.dma_start(out=t_vo, in_=vones[:])
            nc.vector.tensor_copy(
                t_V[:, :, :, DK : DK + 1],
                t_vo[:, :].rearrange("p (kb h one) -> p kb h one", h=HPG, one=1),
            )

            # warm the ACT exp table early (one-time ~2.7us load)
            t_warm = scr.tile([1, 1], F32, name="t_warm", tag="warm", bufs=1)
            nc.scalar.activation(out=t_warm, in_=t_ones[:, 0:1], func=Exp)

            # ---- phase A: projections (v first: its weights arrive first
            # and its SBUF copies finish before attention needs the DVE) ----
            with tc.tile_pool(name="ps_proj", bufs=8, space=PSUM) as ps_proj:
                # v: natural layout [kpos, o] per 128-block.
                for rnd, sts in enumerate(vchunks):
                    w = len(sts) * 128
                    s0 = sts[0] * 128
                    vaccs = [
                        ps_proj.tile([128, OC], F32, tag="proj", name=f"vacc_{rnd}_{n}")
                        for n in range(len(sts))
                    ]
                    for i in range(IT):
                        xt = xs.tile(
                            [128, w], F16, tag="xv", name=f"xtv_{rnd}_{i}", bufs=3
                        )
                        nc.scalar.dma_start(
                            out=xt, in_=xv[i * 128 : (i + 1) * 128, s0 : s0 + w]
                        )
                        for n in range(len(sts)):
                            nc.tensor.matmul(
                                vaccs[n],
                                xt[:, n * 128 : (n + 1) * 128],
                                t_wv[:, i, :],
                                start=(i == 0),
                                stop=False,
                            )
                    for n, st_ in enumerate(sts):
                        nc.tensor.matmul(
                            vaccs[n],
                            t_vrow[:, st_ * 128 : (st_ + 1) * 128],
                            t_bias[:, 2, :],
                            start=False,
                            stop=True,
                        )
                    for n, st_ in enumerate(sts):
                        nc.scalar.copy(
                            t_V[:, st_, :, 0:DK],
                            vaccs[n].rearrange("p (h d) -> p h d", h=HPG),
                        )

                # q and k: output transposed [o, s] (o on partitions).
                for name, xin, w_sb, dst, nsc, bidx in (
                    ("q", xq, t_wq, t_qT, SC, 0),
                    ("k", xk, t_wk, t_kT, ksc, 1),
                ):
                    accs = [
                        ps_proj.tile(
                            [128, 512], F32, tag="proj", name=f"acc_{name}_{n}"
                        )
                        for n in range(OT * nsc)
                    ]
                    for i in range(IT):
                        xt = xs.tile(
                            [128, nsc * 512], F16, tag=f"x{name}",
                            name=f"xt_{name}_{i}", bufs=3,
                        )
                        nc.sync.dma_start(
                            out=xt, in_=xin[i * 128 : (i + 1) * 128, 0 : nsc * 512]
                        )
                        for ot in range(OT):
                            for sc in range(nsc):
                                nc.tensor.matmul(
                                    accs[ot * nsc + sc],
                                    w_sb[:, i, ot * 128 : (ot + 1) * 128],
                                    xt[:, sc * 512 : (sc + 1) * 512],
                                    start=(i == 0),
                                    stop=False,
                                )
                    for ot in range(OT):
                        for sc in range(nsc):
                            nc.tensor.matmul(
                                accs[ot * nsc + sc],
                                t_bias[:, bidx, ot * 128 : (ot + 1) * 128],
                                t_ones,
                                start=False,
                                stop=True,
                            )
                    for ot in range(OT):
                        for sc in range(nsc):
                            nc.scalar.copy(
                                dst[:, ot, sc * 512 : (sc + 1) * 512],
                                accs[ot * nsc + sc],
                            )

            # ---- phase B: attention. Single-pair kb loop with PV lagged
            # two steps behind the scores matmuls (st bufs=3) so the
            # ~1.1us exp latency is fully hidden behind independent PE
            # work; o_ps is drained to SBUF right after the last PV so
            # normalization runs entirely out of SBUF off the critical
            # path. ---- phase C: output projection, deferred. ----
            with (
                tc.tile_pool(name="ps_att", bufs=3, space=PSUM) as ps_att,
                tc.tile_pool(name="ps_o", bufs=1, space=PSUM) as ps_o,
                tc.tile_pool(name="pb", bufs=6) as pb,
                tc.tile_pool(name="nrm", bufs=2) as nrm,
                tc.tile_pool(name="stg", bufs=3) as stg,
            ):
                deferred_norm = []
                deferred_c = []

                def flush(lst):
                    while lst:
                        lst.pop(0)()

                exp_i = [0]

                def emit_exp(stv, ptv_f16, ptv_u16):
                    use_dve = exp_i[0] % 2 == 0
                    exp_i[0] += 1
                    if use_dve:
                        nc.vector.tensor_scalar(
                            out=ptv_u16, in0=stv, scalar1=A_EXP, scalar2=B_EXP,
                            op0=Mult, op1=Add,
                        )
                    else:
                        nc.scalar.activation(out=ptv_f16, in_=stv, func=Exp)

                LAG = 2
                for qc in range(SC):
                    q0 = qc * 512
                    nkb = min(4 * (qc + 1), kb_cap)
                    for ot in range(OT):
                        o_ps = ps_o.tile(
                            [128, 1024], F32, tag="o", name=f"o_{qc}_{ot}", bufs=1
                        )
                        window = []

                        def emit_pv(ent, stop):
                            ppt, poff, pkb = ent
                            for h2 in range(2):
                                nc.tensor.matmul(
                                    o_ps[0 : DK + 1, h2 * 512 + poff : (h2 + 1) * 512],
                                    t_V[:, pkb, 2 * ot + h2, :],
                                    ppt[:, h2 * 512 + poff : (h2 + 1) * 512],
                                    start=(pkb == 0),
                                    stop=stop,
                                )

                        for kb in range(nkb):
                            k0 = kb * 128
                            off = max(0, k0 - q0)
                            st = ps_att.tile(
                                [128, 1024], F32, tag="st",
                                name=f"st_{qc}_{ot}_{kb}", bufs=3,
                            )
                            for h2 in range(2):
                                nc.tensor.matmul(
                                    st[:, h2 * 512 + off : (h2 + 1) * 512],
                                    t_kT[h2 * 64 : (h2 + 1) * 64, ot, k0 : k0 + 128],
                                    t_qT[
                                        h2 * 64 : (h2 + 1) * 64, ot,
                                        q0 + off : q0 + 512,
                                    ],
                                    start=True,
                                    stop=True,
                                )
                            if kb == 1:
                                flush(deferred_norm)
                            if kb >= 3 and kb % 2 == 1 and deferred_c:
                                deferred_c.pop(0)()
                            if len(window) == LAG:
                                emit_pv(window.pop(0), stop=False)
                            pt = pb.tile(
                                [128, 1024], F16, tag="pt",
                                name=f"pt_{qc}_{ot}_{kb}", bufs=6,
                            )
                            if off == 0:
                                emit_exp(st[:, :], pt[:, :], pt[:, :].bitcast(U16))
                            else:
                                stv = st[:, :].rearrange(
                                    "p (two q) -> p two q", two=2
                                )[:, :, off:]
                                ptv = pt[:, :].rearrange(
                                    "p (two q) -> p two q", two=2
                                )[:, :, off:]
                                emit_exp(stv, ptv, ptv.bitcast(U16))
                            if k0 >= q0:
                                for h2 in range(2):
                                    c0 = h2 * 512 + off
                                    nc.gpsimd.affine_select(
                                        out=pt[:, c0 : c0 + 128],
                                        in_=pt[:, c0 : c0 + 128],
                                        compare_op=IsGe,
                                        fill=0.0,
                                        base=0,
                                        channel_multiplier=-1,
                                        pattern=[[1, 128]],
                                    )
                            window.append((pt, off, kb))
                        while len(window) > 1:
                            emit_pv(window.pop(0), stop=False)
                        emit_pv(window.pop(0), stop=True)
                        # drain o (incl. denominator row) to SBUF, freeing the
                        # PSUM bank; norm runs from SBUF later.
                        t_o = nrm.tile(
                            [DK + 1, 1024], F16, tag="to", name=f"to_{qc}_{ot}", bufs=2
                        )
                        nc.scalar.copy(t_o, o_ps[0 : DK + 1, :])

                        def norm(qc=qc, ot=ot, t_o=t_o, q0=q0):
                            t_rp = nrm.tile(
                                [1, 1024], F32, tag="rp", name=f"rp_{qc}_{ot}", bufs=2
                            )
                            t_l32 = nrm.tile(
                                [1, 1024], F32, tag="l32", name=f"l32_{qc}_{ot}", bufs=2
                            )
                            nc.vector.tensor_copy(t_l32, t_o[DK : DK + 1, :])
                            nc.vector.reciprocal_approx_fast(t_rp, t_l32)
                            t_rb = nrm.tile(
                                [DK, 1024], F32, tag="rb", name=f"rb_{qc}_{ot}", bufs=2
                            )
                            nc.gpsimd.partition_broadcast(t_rb, t_rp)
                            for h2 in range(2):
                                nc.vector.tensor_tensor(
                                    out=t_OT[
                                        h2 * 64 : h2 * 64 + DK, ot, q0 : q0 + 512
                                    ],
                                    in0=t_o[0:DK, h2 * 512 : (h2 + 1) * 512],
                                    in1=t_rb[:, h2 * 512 : (h2 + 1) * 512],
                                    op=Mult,
                                )

                        deferred_norm.append(norm)

                    def phase_c_one(qc, q0, dpair):
                        if True:
                            cps = ps_att.tile(
                                [128, 1024], F32, tag="st",
                                name=f"c_{qc}_{dpair}", bufs=3,
                            )
                            for half in range(2):
                                dt = dpair * 2 + half
                                for j in range(OT):
                                    nc.tensor.matmul(
                                        cps[:, half * 512 : (half + 1) * 512],
                                        t_wo[:, j, dt * 128 : (dt + 1) * 128],
                                        t_OT[:, j, q0 : q0 + 512],
                                        start=(j == 0),
                                        stop=(j == OT - 1),
                                    )
                            st_o = stg.tile(
                                [128, 1024], F16, tag="stg",
                                name=f"so_{qc}_{dpair}", bufs=3,
                            )
                            if dpair % 2 == 0:
                                nc.vector.tensor_copy(st_o, cps)
                            else:
                                nc.scalar.copy(st_o, cps)
                            nc.sync.dma_start(
                                out=out_t[
                                    dpair * 256 : (dpair + 1) * 256, q0 : q0 + 512
                                ].rearrange("(two p) q -> p two q", p=128),
                                in_=st_o.rearrange("p (two q) -> p two q", two=2),
                            )

                    for dpair in range(D // 256):
                        deferred_c.append(
                            lambda qc=qc, q0=q0, dpair=dpair: phase_c_one(qc, q0, dpair)
                        )
                flush(deferred_norm)
                flush(deferred_c)
    nc.compile()
    return nc


def _get_nc(kb_cap):
    key = ("nc", kb_cap)
    if key not in _cache:
        _cache[key] = _build_nc(kb_cap)
    return _cache[key]


def kernel(
    query,
    key,
    value,
    Wq,
    bq,
    Wk,
    bk,
    Wv,
    bv,
    Wo,
    bo,
    attn_mask,
    key_padding_mask,
):
    from concourse import bass_utils

    query = np.asarray(query, dtype=np.float32)
    key = np.asarray(key, dtype=np.float32)
    value = np.asarray(value, dtype=np.float32)
    Wq = np.asarray(Wq, dtype=np.float32)
    bq = np.asarray(bq, dtype=np.float32)
    Wk = np.asarray(Wk, dtype=np.float32)
    bk = np.asarray(bk, dtype=np.float32)
    Wv = np.asarray(Wv, dtype=np.float32)
    bv = np.asarray(bv, dtype=np.float32)
    Wo = np.asarray(Wo, dtype=np.float32)
    bo = np.asarray(bo, dtype=np.float32)
    attn_mask = np.asarray(attn_mask)
    key_padding_mask = np.asarray(key_padding_mask)

    # this kernel hardcodes the causal structure of attn_mask
    expected = np.triu(np.ones((S, S), dtype=bool), k=1)
    assert np.array_equal(attn_mask, expected), "kernel assumes causal attn_mask"

    valid = ~key_padding_mask  # [B, S]
    kb_cap = 0
    for b in range(B):
        nz = np.nonzero(valid[b])[0]
        cap = (int(nz.max()) // 128 + 1) if nz.size else 1
        kb_cap = max(kb_cap, cap)

    scale = np.float32(1.0 / np.sqrt(DK))

    xq_b = [np.ascontiguousarray(query[:, b, :].T.astype(np.float16)) for b in range(B)]
    xk_b = [np.ascontiguousarray(key[:, b, :].T.astype(np.float16)) for b in range(B)]
    xv_b = []
    for b in range(B):
        xvb = value[:, b, :].T.copy()
        xvb[:, ~valid[b]] = 0.0
        xv_b.append(np.ascontiguousarray(xvb.astype(np.float16)))
    vrow_b = [
        np.ascontiguousarray(valid[b].astype(np.float16)[None, :]) for b in range(B)
    ]
    vones_b = [
        np.ascontiguousarray(
            np.repeat(
                valid[b][0 : kb_cap * 128].reshape(kb_cap, 128).T[:, :, None],
                HPG,
                axis=2,
            )
            .reshape(128, kb_cap * HPG)
            .astype(np.float16)
        )
        for b in range(B)
    ]
    ones_row = np.ones((1, 512), np.float16)

    def rearr_w(a, nt):
        # [nt*128, X] -> [128, nt*X]
        X = a.shape[1]
        return np.ascontiguousarray(
            a.reshape(nt, 128, X).transpose(1, 0, 2).reshape(128, nt * X)
        )

    in_maps = []
    for c in range(N_CORES):
        b = c // GROUPS
        g = c % GROUPS
        osl = slice(g * OC, (g + 1) * OC)
        bias_cat = np.concatenate([bq[osl] * scale, bk[osl], bv[osl]]).astype(
            np.float16
        )
        brow_cat = np.concatenate([bias_cat, vrow_b[b][0], ones_row[0]])[None, :]
        wrest_cat = np.concatenate(
            [
                rearr_w((Wq[osl, :] * scale).T.astype(np.float16), IT),
                rearr_w(Wk[osl, :].T.astype(np.float16), IT),
                rearr_w(np.ascontiguousarray(Wo[:, osl].T).astype(np.float16), OT),
            ],
            axis=1,
        )
        in_maps.append(
            {
                "xq": xq_b[b],
                "xk": xk_b[b],
                "xv": xv_b[b],
                "wv_d": rearr_w(Wv[osl, :].T.astype(np.float16), IT),
                "wrest": np.ascontiguousarray(wrest_cat),
                "brow": np.ascontiguousarray(brow_cat),
                "vones": vones_b[b],
            }
        )

    res = bass_utils.run_bass_kernel_spmd(
        _get_nc(kb_cap), in_maps, core_ids=list(range(N_CORES))
    )
    _cache["last_res"] = res

    out = np.zeros((S, B, D), dtype=np.float32)
    for b in range(B):
        acc = np.zeros((D, S), dtype=np.float32)
        for g in range(GROUPS):
            acc += res.results[b * GROUPS + g]["out_t"].astype(np.float32)
        out[:, b, :] = acc.T + bo[None, :]
    return out
